# revision 20
# baseline (speedup 1.0000x reference)
"""Trainium2 Bass kernel for nn_NeuralQuantizer (vq_codebook).

reference semantics (fp32):
    idx = argmin_i |x - centers_i|   (first-min tie break)
    out = x + stop_gradient(centers[idx] - x)  == centers[idx] in forward

centers = jnp.linspace(-1, 1, 256) is a UNIFORM grid, so the argmin
collapses to an affine round:

    b = clamp(round_ne(127.5*x + 127.5), 0, 255)
    out = b * (2/255) - 1

The whole computation is ONE fused custom-DVE op (8 ALU stages):

    h = (minn(relu(Src0*C0 + C0), C0 + C0) + C1 - C1) * C2 - One

with C0 = 127.5 (s0), C1 = 1.5*2^23 (s1, round-to-nearest-even magic),
C2 = 2/255 (imm2).  `C0 + C0` (= 255, the clamp ceiling) is a
stream-invariant subexpression that lower() hoists at zero stage cost,
which is what makes everything fit in 3 scalar slots / 8 stages.

Numerics vs the bit-exact reference (measured on the actual test
input): rel err 2.6e-5 (tolerance 2e-2).  Differences are last-ulp
dequantize rounding plus a handful of one-step boundary ties.

Per core: 1 MiB in + 1 MiB out as four contiguous 256 KiB HBM tiles
(flat [1, N] DRAM declaration), DMAs alternating between the SP and
ACT HWDGE rings so input and output streams issue concurrently and the
SDMA engines interleave them at packet granularity (measured 380-400
GB/s aggregate during the overlap).  DVE total busy ~2.8 us, fully
hidden under the DMA streams.

Measured structure of the ~18 us exec time (profiled on HW):
  ~7.2 us fixed preamble (host start doorbell ~3.3, engine table loads
          ~1.4, barriers/memsets; first DMA issue is always ~7.2),
  ~1.5 us HWDGE issue->first-packet latency,
  ~6.5 us streaming window (2 MiB at ~390 GB/s + pipeline bubble),
  ~2.8 us tail (last-DMA receipt + profile epilogue; exec_time_ns
          empirically = last-DMA-packet-end + 2.77 us).
"""

import numpy as np

N_CORES = 8
SHAPE = (4, 512, 1024)
TOTAL = SHAPE[0] * SHAPE[1] * SHAPE[2]          # 2097152
PER_CORE = TOTAL // N_CORES                     # 262144
P = 128                                         # SBUF partitions
FD = PER_CORE // P                              # 2048 floats per partition

MAGIC = 12582912.0                              # 1.5 * 2**23
R2 = float(np.float32(2.0) / np.float32(255.0))

# Tunables (experiment config; defaults = current best known: ~18.0 us
# median, ~17.5 best over many HW reps; run-to-run noise is +-1 us)
CFG = {
    "nt": 4,             # tiles along the free dim (ignored if splits given)
    "splits": None,      # explicit tile widths summing to FD, e.g. [512, 1536]
    "bufs": 4,           # tile pool depth
    "out_dma": "scalar,sync",  # cycle of HWDGE rings for out DMAs
    "in_dma": "sync,scalar",   # cycle of rings for in DMAs
    "layout": "flat",    # "col": x=[P,FD], tiles slice columns (strided HBM)
                         # "row": x=[nt*P,tfd], each tile a contiguous HBM block
                         # "flat": x=[1,N], tiles contiguous, uneven sizes OK
    "impl": "custom",    # "custom": 1 fused DVE op; "stock": 4 tensor_scalar ops
    "split_last_out": 2, # 2 = final out as two partition-half DMAs on both rings
}

_cache = {}


def _register_vq_ops():
    """Register the fused quantize-dequantize as one custom DVE op
    (appended to dve_ops.OPS, the documented extension point).

      VQDQ_ANT(x) = (minn(relu(x*C0 + C0), C0+C0) + C1 - C1) * C2 - 1

    i.e. b = round_ne(clamp(127.5x + 127.5, 0, 255)); out = b*(2/255) - 1.
    Single tensor stream, 3 scalar constants, 8 ALU stages.
    """
    import concourse.dve_ops as dom
    from concourse.dve_ops import DveOp
    from concourse.dve_spec import (
        Spec, Src0, C0, C1, C2, One, relu, minn, lower, _has_src1,
    )
    from concourse.dve_uop import DveOpSpec

    if "VQDQ_ANT" in dom._SUB_OPCODE_FOR_NAME:
        return

    f32 = np.float32

    def _ref(in0, in1, s0, s1, imm2):
        a = (in0 * f32(s0)).astype(f32)
        b = (a + f32(s0)).astype(f32)
        c = np.maximum(b, f32(0)).astype(f32)
        d = np.minimum(c, (f32(s0) + f32(s0)).astype(f32)).astype(f32)
        e = (d + f32(s1)).astype(f32)
        f = (e - f32(s1)).astype(f32)
        g = (f * f32(imm2)).astype(f32)
        return (g - f32(1)).astype(f32)

    a = Src0 * C0
    b = a + C0
    c = relu(b)
    d = minn(c, C0 + C0)
    e = d + C1
    f = e - C1
    g = f * C2
    body = g - One

    spec = Spec(body=body, reference=_ref)
    row = dom._CUSTOM_DVE_ROW_BASE + len(dom.OPS)
    assert row < 0x20
    uops = lower(spec, ver="v3")
    sha = DveOpSpec(
        name="VQDQ_ANT", opcode=row, uops=uops, rd1_en=_has_src1(spec)
    ).sha("v3")
    op = DveOp("VQDQ_ANT", spec, subdim=False, uops_sha={"v3": sha})
    dom.OPS.append(op)
    dom._SUB_OPCODE_FOR_NAME["VQDQ_ANT"] = row
    dom.CUSTOM_DVE_SPECS["VQDQ_ANT"] = spec


def _build(cfg=None):
    import concourse.bacc as bacc
    import concourse.mybir as mybir
    from concourse.tile import TileContext

    cfg = dict(CFG, **(cfg or {}))
    splits = cfg["splits"] or [FD // cfg["nt"]] * cfg["nt"]
    assert sum(splits) == FD, splits
    vq_op = None
    if cfg["impl"] == "custom":
        _register_vq_ops()
        import concourse.dve_ops as dom
        vq_op = next(o for o in dom.OPS if o.name == "VQDQ_ANT")

    f32 = mybir.dt.float32
    layout = cfg["layout"]
    if layout == "row":
        nt = len(splits)
        tfd0 = splits[0]
        assert all(s == tfd0 for s in splits), "row layout needs equal splits"

    # Bacc (not raw Bass): its compile() pass splits multi-sem waits into
    # event semaphores -- TRN2 instructions carry at most one sync wait.
    nc = bacc.Bacc()
    if layout == "row":
        x_in = nc.declare_dram_parameter("x", [nt * P, tfd0], f32, isOutput=False)
        y_out = nc.declare_dram_parameter("y", [nt * P, tfd0], f32, isOutput=True)
    elif layout == "flat":
        x_in = nc.declare_dram_parameter("x", [1, PER_CORE], f32, isOutput=False)
        y_out = nc.declare_dram_parameter("y", [1, PER_CORE], f32, isOutput=True)
    else:
        x_in = nc.declare_dram_parameter("x", [P, FD], f32, isOutput=False)
        y_out = nc.declare_dram_parameter("y", [P, FD], f32, isOutput=True)

    eng = {"sync": nc.sync, "scalar": nc.scalar, "gpsimd": nc.gpsimd}
    in_rings = [eng[e].dma_start for e in cfg["in_dma"].split(",")]
    out_rings = [eng[e].dma_start for e in cfg["out_dma"].split(",")]

    with TileContext(nc) as tc:
        with tc.tile_pool(name="pool", bufs=cfg["bufs"]) as pool:
            off = 0
            for it, tfd in enumerate(splits):
                if layout == "row":
                    src = x_in[it * P:(it + 1) * P, :]
                    dst = y_out[it * P:(it + 1) * P, :]
                elif layout == "flat":
                    sl = slice(off * P, (off + tfd) * P)
                    off += tfd
                    src = x_in[0, sl]
                    dst = y_out[0, sl]
                else:
                    sl = slice(off, off + tfd)
                    off += tfd
                    src = x_in[:, sl]
                    dst = y_out[:, sl]
                xs = pool.tile([P, tfd], f32, tag=f"xs{it}")
                in_rings[it % len(in_rings)](out=xs[:], in_=src)
                q = pool.tile([P, tfd], f32, tag=f"q{it}")
                if cfg["impl"] == "custom":
                    nc.vector._custom_dve(vq_op, out=q[:], in0=xs[:],
                                          s0=127.5, s1=MAGIC, imm2=R2)
                else:
                    op = mybir.AluOpType
                    # (x*127.5+127.5) -> clamp[0,255] -> round_ne -> *R2-1
                    w = pool.tile([P, tfd], f32, tag=f"w{it}")
                    nc.vector.tensor_scalar(w[:], xs[:], 127.5, 127.5,
                                            op.mult, op.add)
                    c = pool.tile([P, tfd], f32, tag=f"c{it}")
                    nc.vector.tensor_scalar(c[:], w[:], 0.0, 255.0,
                                            op.max, op.min)
                    b = pool.tile([P, tfd], f32, tag=f"b{it}")
                    nc.vector.tensor_scalar(b[:], c[:], MAGIC, MAGIC,
                                            op.add, op.subtract)
                    nc.vector.tensor_scalar(q[:], b[:], R2, 1.0,
                                            op.mult, op.subtract)
                if (cfg["split_last_out"] == 2 and it == len(splits) - 1
                        and layout == "flat"):
                    # Final out as two partition-half DMAs issued concurrently
                    # on both HWDGE rings: halves hit disjoint SDMA engine
                    # sets (ports swizzle even/odd by partition), so their
                    # streams overlap and the tail stream time halves.
                    base = sl.start
                    half = (tfd * P) // 2
                    eng["sync"].dma_start(out=y_out[0, base:base + half],
                                          in_=q[:P // 2, :])
                    eng["scalar"].dma_start(out=y_out[0, base + half:sl.stop],
                                            in_=q[P // 2:, :])
                else:
                    out_rings[it % len(out_rings)](out=dst, in_=q[:])

    nc.finalize()
    return nc


def _get_nc(cfg=None):
    key = repr(sorted(dict(CFG, **(cfg or {})).items()))
    if key not in _cache:
        _cache[key] = _build(cfg)
    return _cache[key]


def _shard_shape(cfg=None):
    cfg = dict(CFG, **(cfg or {}))
    if cfg["layout"] == "row":
        splits = cfg["splits"] or [FD // cfg["nt"]] * cfg["nt"]
        return (len(splits) * P, splits[0])
    if cfg["layout"] == "flat":
        return (1, PER_CORE)
    return (P, FD)


def kernel(x, centers=None):
    from concourse.bass_utils import run_bass_kernel_spmd

    shp = _shard_shape()
    x = np.ascontiguousarray(np.asarray(x, dtype=np.float32))
    flat = x.reshape(-1)
    shards = [
        np.ascontiguousarray(flat[i * PER_CORE:(i + 1) * PER_CORE].reshape(shp))
        for i in range(N_CORES)
    ]
    in_maps = [{"x": s} for s in shards]
    nc = _get_nc()
    res = run_bass_kernel_spmd(nc, in_maps, core_ids=list(range(N_CORES)))
    out = np.concatenate([res.results[i]["y"].reshape(-1) for i in range(N_CORES)])
    return out.reshape(SHAPE).astype(np.float32)


# revision 22
# speedup vs baseline: 1.1973x; 1.1973x over previous
"""Trainium2 Bass kernel for nn_NeuralQuantizer (vq_codebook).

reference semantics (fp32):
    idx = argmin_i |x - centers_i|   (first-min tie break)
    out = x + stop_gradient(centers[idx] - x)  == centers[idx] in forward

centers = jnp.linspace(-1, 1, 256) is a UNIFORM grid, so the argmin
collapses to an affine round:

    b = clamp(round_ne(127.5*x + 127.5), 0, 255)
    out = b * (2/255) - 1

The whole computation is ONE fused custom-DVE op (8 ALU stages):

    h = (minn(relu(Src0*C0 + C0), C0 + C0) + C1 - C1) * C2 - One

with C0 = 127.5 (s0), C1 = 1.5*2^23 (s1, round-to-nearest-even magic),
C2 = 2/255 (imm2).  `C0 + C0` (= 255, the clamp ceiling) is a
stream-invariant subexpression that lower() hoists at zero stage cost,
which is what makes everything fit in 3 scalar slots / 8 stages.

Numerics vs the bit-exact reference (measured on the actual test
input): rel err 2.6e-5 (tolerance 2e-2).  Differences are last-ulp
dequantize rounding plus a handful of one-step boundary ties.

Per core: 1 MiB in + 1 MiB out as four contiguous 256 KiB HBM tiles
(flat [1, N] DRAM declaration), DMAs alternating between the SP and
ACT HWDGE rings so input and output streams issue concurrently and the
SDMA engines interleave them at packet granularity (measured 380-400
GB/s aggregate during the overlap).  DVE total busy ~2.8 us, fully
hidden under the DMA streams.

Measured structure of the ~18 us exec time (profiled on HW):
  ~7.2 us fixed preamble (host start doorbell ~3.3, engine table loads
          ~1.4, barriers/memsets; first DMA issue is always ~7.2),
  ~1.5 us HWDGE issue->first-packet latency,
  ~6.5 us streaming window (2 MiB at ~390 GB/s + pipeline bubble),
  ~2.8 us tail (last-DMA receipt + profile epilogue; exec_time_ns
          empirically = last-DMA-packet-end + 2.77 us).
"""

import numpy as np

N_CORES = 8
SHAPE = (4, 512, 1024)
TOTAL = SHAPE[0] * SHAPE[1] * SHAPE[2]          # 2097152
PER_CORE = TOTAL // N_CORES                     # 262144
P = 128                                         # SBUF partitions
FD = PER_CORE // P                              # 2048 floats per partition

MAGIC = 12582912.0                              # 1.5 * 2**23
R2 = float(np.float32(2.0) / np.float32(255.0))

# Tunables (experiment config; defaults = current best known: ~18.0 us
# median, ~17.5 best over many HW reps; run-to-run noise is +-1 us)
CFG = {
    "nt": 4,             # tiles along the free dim (ignored if splits given)
    "splits": None,      # explicit tile widths summing to FD, e.g. [512, 1536]
    "bufs": 4,           # tile pool depth
    "out_dma": "sync",         # all outs on warm Q1 ring: skips the ~0.5-1 us
                               # ACT-ring (Q10) first-packet warmup on out0;
                               # won 3/3 interleaved pairs vs alternating rings
    "in_dma": "sync,scalar",   # cycle of rings for in DMAs
    "layout": "flat",    # "col": x=[P,FD], tiles slice columns (strided HBM)
                         # "row": x=[nt*P,tfd], each tile a contiguous HBM block
                         # "flat": x=[1,N], tiles contiguous, uneven sizes OK
    "impl": "custom",    # "custom": 1 fused DVE op; "stock": 4 tensor_scalar ops
    "split_last_out": 1, # 2 = final out as two partition-half DMAs on both rings
                         # (kept off: its two fresh-process runs were the worst
                         # samples of the series; warm-run gain only ~0.3 us)
}

_cache = {}


def _register_vq_ops():
    """Register the fused quantize-dequantize as one custom DVE op
    (appended to dve_ops.OPS, the documented extension point).

      VQDQ_ANT(x) = (minn(relu(x*C0 + C0), C0+C0) + C1 - C1) * C2 - 1

    i.e. b = round_ne(clamp(127.5x + 127.5, 0, 255)); out = b*(2/255) - 1.
    Single tensor stream, 3 scalar constants, 8 ALU stages.
    """
    import concourse.dve_ops as dom
    from concourse.dve_ops import DveOp
    from concourse.dve_spec import (
        Spec, Src0, C0, C1, C2, One, relu, minn, lower, _has_src1,
    )
    from concourse.dve_uop import DveOpSpec

    if "VQDQ_ANT" in dom._SUB_OPCODE_FOR_NAME:
        return

    f32 = np.float32

    def _ref(in0, in1, s0, s1, imm2):
        a = (in0 * f32(s0)).astype(f32)
        b = (a + f32(s0)).astype(f32)
        c = np.maximum(b, f32(0)).astype(f32)
        d = np.minimum(c, (f32(s0) + f32(s0)).astype(f32)).astype(f32)
        e = (d + f32(s1)).astype(f32)
        f = (e - f32(s1)).astype(f32)
        g = (f * f32(imm2)).astype(f32)
        return (g - f32(1)).astype(f32)

    a = Src0 * C0
    b = a + C0
    c = relu(b)
    d = minn(c, C0 + C0)
    e = d + C1
    f = e - C1
    g = f * C2
    body = g - One

    spec = Spec(body=body, reference=_ref)
    row = dom._CUSTOM_DVE_ROW_BASE + len(dom.OPS)
    assert row < 0x20
    uops = lower(spec, ver="v3")
    sha = DveOpSpec(
        name="VQDQ_ANT", opcode=row, uops=uops, rd1_en=_has_src1(spec)
    ).sha("v3")
    op = DveOp("VQDQ_ANT", spec, subdim=False, uops_sha={"v3": sha})
    dom.OPS.append(op)
    dom._SUB_OPCODE_FOR_NAME["VQDQ_ANT"] = row
    dom.CUSTOM_DVE_SPECS["VQDQ_ANT"] = spec


def _build(cfg=None):
    import concourse.bacc as bacc
    import concourse.mybir as mybir
    from concourse.tile import TileContext

    cfg = dict(CFG, **(cfg or {}))
    splits = cfg["splits"] or [FD // cfg["nt"]] * cfg["nt"]
    assert sum(splits) == FD, splits
    vq_op = None
    if cfg["impl"] == "custom":
        _register_vq_ops()
        import concourse.dve_ops as dom
        vq_op = next(o for o in dom.OPS if o.name == "VQDQ_ANT")

    f32 = mybir.dt.float32
    layout = cfg["layout"]
    if layout == "row":
        nt = len(splits)
        tfd0 = splits[0]
        assert all(s == tfd0 for s in splits), "row layout needs equal splits"

    # Bacc (not raw Bass): its compile() pass splits multi-sem waits into
    # event semaphores -- TRN2 instructions carry at most one sync wait.
    nc = bacc.Bacc()
    if layout == "row":
        x_in = nc.declare_dram_parameter("x", [nt * P, tfd0], f32, isOutput=False)
        y_out = nc.declare_dram_parameter("y", [nt * P, tfd0], f32, isOutput=True)
    elif layout == "flat":
        x_in = nc.declare_dram_parameter("x", [1, PER_CORE], f32, isOutput=False)
        y_out = nc.declare_dram_parameter("y", [1, PER_CORE], f32, isOutput=True)
    else:
        x_in = nc.declare_dram_parameter("x", [P, FD], f32, isOutput=False)
        y_out = nc.declare_dram_parameter("y", [P, FD], f32, isOutput=True)

    eng = {"sync": nc.sync, "scalar": nc.scalar, "gpsimd": nc.gpsimd}
    in_rings = [eng[e].dma_start for e in cfg["in_dma"].split(",")]
    out_rings = [eng[e].dma_start for e in cfg["out_dma"].split(",")]

    with TileContext(nc) as tc:
        with tc.tile_pool(name="pool", bufs=cfg["bufs"]) as pool:
            off = 0
            for it, tfd in enumerate(splits):
                if layout == "row":
                    src = x_in[it * P:(it + 1) * P, :]
                    dst = y_out[it * P:(it + 1) * P, :]
                elif layout == "flat":
                    sl = slice(off * P, (off + tfd) * P)
                    off += tfd
                    src = x_in[0, sl]
                    dst = y_out[0, sl]
                else:
                    sl = slice(off, off + tfd)
                    off += tfd
                    src = x_in[:, sl]
                    dst = y_out[:, sl]
                xs = pool.tile([P, tfd], f32, tag=f"xs{it}")
                in_rings[it % len(in_rings)](out=xs[:], in_=src)
                q = pool.tile([P, tfd], f32, tag=f"q{it}")
                if cfg["impl"] == "custom":
                    nc.vector._custom_dve(vq_op, out=q[:], in0=xs[:],
                                          s0=127.5, s1=MAGIC, imm2=R2)
                else:
                    op = mybir.AluOpType
                    # (x*127.5+127.5) -> clamp[0,255] -> round_ne -> *R2-1
                    w = pool.tile([P, tfd], f32, tag=f"w{it}")
                    nc.vector.tensor_scalar(w[:], xs[:], 127.5, 127.5,
                                            op.mult, op.add)
                    c = pool.tile([P, tfd], f32, tag=f"c{it}")
                    nc.vector.tensor_scalar(c[:], w[:], 0.0, 255.0,
                                            op.max, op.min)
                    b = pool.tile([P, tfd], f32, tag=f"b{it}")
                    nc.vector.tensor_scalar(b[:], c[:], MAGIC, MAGIC,
                                            op.add, op.subtract)
                    nc.vector.tensor_scalar(q[:], b[:], R2, 1.0,
                                            op.mult, op.subtract)
                if (cfg["split_last_out"] == 2 and it == len(splits) - 1
                        and layout == "flat"):
                    # Final out as two partition-half DMAs issued concurrently
                    # on both HWDGE rings: halves hit disjoint SDMA engine
                    # sets (ports swizzle even/odd by partition), so their
                    # streams overlap and the tail stream time halves.
                    base = sl.start
                    half = (tfd * P) // 2
                    eng["sync"].dma_start(out=y_out[0, base:base + half],
                                          in_=q[:P // 2, :])
                    eng["scalar"].dma_start(out=y_out[0, base + half:sl.stop],
                                            in_=q[P // 2:, :])
                else:
                    out_rings[it % len(out_rings)](out=dst, in_=q[:])

    nc.finalize()
    return nc


def _get_nc(cfg=None):
    key = repr(sorted(dict(CFG, **(cfg or {})).items()))
    if key not in _cache:
        _cache[key] = _build(cfg)
    return _cache[key]


def _shard_shape(cfg=None):
    cfg = dict(CFG, **(cfg or {}))
    if cfg["layout"] == "row":
        splits = cfg["splits"] or [FD // cfg["nt"]] * cfg["nt"]
        return (len(splits) * P, splits[0])
    if cfg["layout"] == "flat":
        return (1, PER_CORE)
    return (P, FD)


def kernel(x, centers=None):
    from concourse.bass_utils import run_bass_kernel_spmd

    shp = _shard_shape()
    x = np.ascontiguousarray(np.asarray(x, dtype=np.float32))
    flat = x.reshape(-1)
    shards = [
        np.ascontiguousarray(flat[i * PER_CORE:(i + 1) * PER_CORE].reshape(shp))
        for i in range(N_CORES)
    ]
    in_maps = [{"x": s} for s in shards]
    nc = _get_nc()
    res = run_bass_kernel_spmd(nc, in_maps, core_ids=list(range(N_CORES)))
    out = np.concatenate([res.results[i]["y"].reshape(-1) for i in range(N_CORES)])
    return out.reshape(SHAPE).astype(np.float32)
